# revision 4
# baseline (speedup 1.0000x reference)
"""GPT-OSS MoE experts kernel for Trainium2 (8 NeuronCores, expert-parallel).

Strategy
--------
- Expert-parallel: core e owns expert e's weights (1/8 of total weight bytes).
- Host does routing (gather tokens per expert), weight re-staging (slice the
  expert, transpose to [K, N] contraction-major layout, pad K to a multiple of
  128, cast to fp16), and the final scatter-add combine. No collectives.
- The reference's per-32-block fp8 quant-dequant collapses exactly to
  "round each element to 4 significant bits (RTNE)": the block scale is a
  power of two (mantissa rounding is scale-invariant) and the +-448 clip can
  never bind by construction. Verified numerically; residual differences are
  confined to the e4m3-subnormal range (~2^-9 * block scale, negligible).
  On device this is 3 VectorE ops (Veltkamp split); on host, the same numpy.
  The 4-significant-bit activations are then EXACT in fp16.
- fp16 weights round at 2^-11; end-to-end error vs the f32 reference is
  ~6e-3 absmax-rel (vs ~5e-3 for full fp32/fp22 weights) — the error budget
  is dominated by quantization-boundary flips from layer-1 perturbations
  either way, and fp16 halves the DMA-bound kernel's weight traffic.
- Biases ride free inside the GEMM: contraction padded 2880 -> 2944,
  activations carry a constant-1 row at index 2880, weights the bias row.
- Weight slabs are the moving operand (N=480, full rate), tiny transposed
  activations are the stationary operand, so layer-1 output lands
  token-major: swiglu is pure elementwise work, and the tiny intermediate is
  PE-transposed on chip to feed layer 2.
"""

import functools
import sys

sys.path.insert(0, "/opt/trn_rl_repo")

import numpy as np

import concourse.bass as bass  # noqa: F401
import concourse.mybir as mybir
import concourse.tile as tile
from concourse import bacc
from concourse.bass_utils import run_bass_kernel_spmd

P = 128
H = 2880          # hidden dim (= layer-1 contraction, layer-2 output)
II = 2880         # intermediate dim (gate/up width, layer-2 contraction)
NE = 8            # experts == cores
KT = 23           # k-tiles of 128 over the padded contraction dim
KP = KT * P       # 2944 = 2880 + 64 pad (row 2880 = ones/bias row)
NB = 6            # n-blocks per 2880-wide output
NBW = H // NB     # 480
VC = float(2 ** 20 + 1)            # Veltkamp constant: RTNE to 4 sig bits

f32 = mybir.dt.float32
f16 = mybir.dt.float16
AF = mybir.ActivationFunctionType
ALU = mybir.AluOpType


def _rtne4(x: np.ndarray) -> np.ndarray:
    """Round f32 elements to 4 significant bits, RTNE (== reference
    quant_dequant_fp8 up to e4m3-subnormal leftovers)."""
    c = np.float32(VC)
    t = (x * c).astype(np.float32)
    return (t - (t - x)).astype(np.float32)


@functools.lru_cache(maxsize=4)
def _build(cgs: tuple[int, ...]):
    """Build the per-core Bass program. cgs = token-group sizes (each <=128)."""
    from concourse.masks import make_identity

    nc = bacc.Bacc(None, target_bir_lowering=False)
    ccap = sum(cgs)

    xt_d = nc.declare_dram_parameter("xt", [KP, ccap], f16, isOutput=False)
    w1g_d = nc.declare_dram_parameter("w1g", [KP, II], f16, isOutput=False)
    w1u_d = nc.declare_dram_parameter("w1u", [KP, II], f16, isOutput=False)
    w2_d = nc.declare_dram_parameter("w2", [KP, H], f16, isOutput=False)
    wr_d = nc.declare_dram_parameter("wr", [ccap, 1], f32, isOutput=False)
    y_d = nc.declare_dram_parameter("y", [ccap, H], f32, isOutput=True)

    w1g_r = w1g_d[:].rearrange("(o p) n -> p o n", p=P)
    w1u_r = w1u_d[:].rearrange("(o p) n -> p o n", p=P)
    w2_r = w2_d[:].rearrange("(o p) n -> p o n", p=P)

    with tile.TileContext(nc) as tc:
        with (
            tc.tile_pool(name="consts", bufs=1) as consts,
            tc.tile_pool(name="wslab", bufs=10) as wpool,
            tc.tile_pool(name="tmp", bufs=2) as tmp,
            tc.tile_pool(name="psum", bufs=2, space="PSUM") as psum,
            tc.tile_pool(name="tpsum", bufs=2, space="PSUM") as tpsum,
        ):
            ident = consts.tile([P, P], f32, tag="ident", name="ident")
            make_identity(nc, ident)

            # HAM warmup: dummy matmuls while the first weight slabs stream in,
            # so the real stream starts at the 2.4 GHz clock.
            wup = tpsum.tile([P, P], f32, tag="tp", name="wup")
            for _ in range(24):
                nc.tensor.matmul(wup, ident, ident, start=True, stop=True,
                                 skip_group_check=True)

            # resident per-group tensors
            xts, wrs, inter, interT = [], [], [], []
            col0 = 0
            for g, cg in enumerate(cgs):
                t = consts.tile([P, KT, cg], f16, tag=f"xt{g}", name=f"xt{g}")
                nc.sync.dma_start(
                    t, xt_d[:, col0 : col0 + cg].rearrange("(o p) c -> p o c", p=P)
                )
                xts.append(t)
                w = consts.tile([cg, 1], f32, tag=f"wr{g}", name=f"wr{g}")
                nc.sync.dma_start(w, wr_d[col0 : col0 + cg, :])
                wrs.append(w)
                it = consts.tile([cg, KP], f32, tag=f"inter{g}", name=f"inter{g}")
                nc.vector.memset(it[:, II : II + 1], 1.0)      # ones row (bias)
                nc.vector.memset(it[:, II + 1 :], 0.0)         # K padding
                inter.append(it)
                itT = consts.tile([P, KT, cg], f16, tag=f"interT{g}",
                                  name=f"interT{g}")
                interT.append(itT)
                col0 += cg

            KC = [(0, 12), (12, 23)]   # slab DMA k-chunks

            def load_slab(src_r, b):
                s = wpool.tile([P, 12, NBW], f16, tag="wslab", name="wslab")
                s2 = wpool.tile([P, 12, NBW], f16, tag="wslab", name="wslab2")
                for (a, bb), t in zip(KC, (s, s2)):
                    nc.sync.dma_start(
                        t[:, : bb - a, :], src_r[:, a:bb, b * NBW : (b + 1) * NBW]
                    )
                return (s, s2)

            def slab_k(slab, k):
                ci = 0 if k < 12 else 1
                return slab[ci][:, k - KC[ci][0], :]

            # ---- layer 1 + swiglu + rtne4 (+ inline transposes) ----
            cg0 = cgs[0]
            ndone = 0   # transposed 128-chunks of inter emitted so far

            def alloc_gu_psums(g, cg):
                # group 0 gets its own banks; later (small) groups share one
                # bank via the column-tiling base-partition trick.
                if g == 0:
                    gps = psum.tile([P, NBW], f32, tag="ps_g0", name="ps_g0")[:cg]
                    ups = psum.tile([P, NBW], f32, tag="ps_u0", name="ps_u0")[:cg]
                else:
                    both = psum.tile([2 * cg, NBW], f32, tag=f"ps_gu{g}",
                                     name=f"ps_gu{g}")
                    gps, ups = both[:cg], both[cg:]
                return gps, ups

            for b in range(NB):
                slab_g = load_slab(w1g_r, b)
                slab_u = load_slab(w1u_r, b)
                for g, cg in enumerate(cgs):
                    gps, ups = alloc_gu_psums(g, cg)
                    for k in range(KT):
                        nc.tensor.matmul(gps, xts[g][:, k, :], slab_k(slab_g, k),
                                         start=(k == 0), stop=(k == KT - 1))
                        nc.tensor.matmul(ups, xts[g][:, k, :], slab_k(slab_u, k),
                                         start=(k == 0), stop=(k == KT - 1))
                    # swiglu: gate=min(G,7); up1=clip(U,-7,7)+1; x=gate*sig(1.702g)*up1
                    gate = tmp.tile([cg0, NBW], f32, tag="t_gate", name="t_gate")[:cg]
                    nc.vector.tensor_scalar_min(gate, gps, 7.0)
                    sig = tmp.tile([cg0, NBW], f32, tag="t_sig", name="t_sig")[:cg]
                    nc.scalar.activation(sig, gate, AF.Sigmoid, scale=1.702)
                    up1 = tmp.tile([cg0, NBW], f32, tag="t_up", name="t_up")[:cg]
                    nc.vector.tensor_scalar(up1, ups, 1.0, -6.0, ALU.add, ALU.max)
                    nc.vector.tensor_scalar_min(up1, up1, 8.0)
                    nc.vector.tensor_mul(gate, gate, sig)          # gate*sig
                    xv = tmp.tile([cg0, NBW], f32, tag="t_xv", name="t_xv")[:cg]
                    nc.vector.tensor_mul(xv, gate, up1)            # x = swiglu
                    tv = tmp.tile([cg0, NBW], f32, tag="t_tv", name="t_tv")[:cg]
                    nc.vector.tensor_scalar_mul(tv, xv, VC)        # t = x*c
                    nc.vector.tensor_sub(xv, tv, xv)               # d = t-x
                    nc.vector.tensor_sub(                          # rtne4 = t-d
                        inter[g][:, b * NBW : (b + 1) * NBW], tv, xv
                    )
                # transpose the 128-chunks completed by this n-block
                ready = ((b + 1) * NBW) // P if b < NB - 1 else KT
                while ndone < ready:
                    i = ndone
                    for g, cg in enumerate(cgs):
                        tp = tpsum.tile([P, cg0], f32, tag="tp", name="tp")[:, :cg]
                        nc.tensor.transpose(
                            tp, inter[g][:, i * P : (i + 1) * P], ident[:cg, :cg]
                        )
                        nc.vector.tensor_copy(interT[g][:, i, :], tp)
                    ndone += 1

            # ---- layer 2 + routing-weight scale ----
            for hb in range(NB):
                slab2 = load_slab(w2_r, hb)
                row0 = 0
                for g, cg in enumerate(cgs):
                    if g == 0:
                        yps = psum.tile([P, NBW], f32, tag="ps_g0", name="ps_g0")[:cg]
                    else:
                        yps = psum.tile([2 * cg, NBW], f32, tag=f"ps_gu{g}",
                                        name=f"ps_gu{g}")[:cg]
                    for k in range(KT):
                        nc.tensor.matmul(yps, interT[g][:, k, :], slab_k(slab2, k),
                                         start=(k == 0), stop=(k == KT - 1))
                    ysb = tmp.tile([cg0, NBW], f32, tag="t_gate", name="t_gate")[:cg]
                    nc.vector.tensor_scalar_mul(ysb, yps, wrs[g])
                    nc.sync.dma_start(
                        y_d[row0 : row0 + cg, hb * NBW : (hb + 1) * NBW], ysb
                    )
                    row0 += cg

    nc.finalize()
    return nc


def _plan_groups(maxc: int) -> tuple[int, ...]:
    ng = max(1, -(-maxc // P))
    last = maxc - P * (ng - 1)
    last = min(P, max(32, -(-last // 32) * 32))
    return tuple([P] * (ng - 1) + [last])


def _stage(inputs):
    """Host-side routing + weight re-staging. Returns (nc, in_maps, assigns, T)."""
    hs = np.ascontiguousarray(np.asarray(inputs["hidden_states"], dtype=np.float32))
    ri = np.asarray(inputs["router_indices"]).astype(np.int64)
    rw = np.asarray(inputs["routing_weights"], dtype=np.float32)
    gup = np.asarray(inputs["gate_up_proj"], dtype=np.float32)
    gub = np.asarray(inputs["gate_up_proj_bias"], dtype=np.float32)
    dn = np.asarray(inputs["down_proj"], dtype=np.float32)
    dnb = np.asarray(inputs["down_proj_bias"], dtype=np.float32)

    T = hs.shape[0]
    topk = ri.shape[1]

    flat_e = ri.reshape(-1)
    order = np.argsort(flat_e, kind="stable")
    counts = np.bincount(flat_e, minlength=NE)
    starts = np.zeros(NE + 1, np.int64)
    starts[1:] = np.cumsum(counts)
    cgs = _plan_groups(int(counts.max()))
    ccap = sum(cgs)

    x_dq = _rtne4(hs).astype(np.float16)   # 4-sig-bit values: exact in fp16
    rw_flat = rw.reshape(-1)

    in_maps, assigns = [], []
    for e in range(NE):
        a = order[starts[e] : starts[e + 1]]
        toks = a // topk
        ce = len(a)
        assigns.append((a, toks))

        xt = np.zeros((KP, ccap), np.float16)
        xt[:H, :ce] = x_dq[toks].T
        xt[H, :] = np.float16(1.0)

        w1g = np.zeros((KP, II), np.float16)
        w1g[:H, :] = gup[e, 0::2, :].T.astype(np.float16)
        w1g[H, :] = gub[e, 0::2].astype(np.float16)
        w1u = np.zeros((KP, II), np.float16)
        w1u[:H, :] = gup[e, 1::2, :].T.astype(np.float16)
        w1u[H, :] = gub[e, 1::2].astype(np.float16)
        w2 = np.zeros((KP, H), np.float16)
        w2[:II, :] = dn[e].T.astype(np.float16)
        w2[II, :] = dnb[e].astype(np.float16)

        wr_col = np.zeros((ccap, 1), np.float32)
        wr_col[:ce, 0] = rw_flat[a]

        in_maps.append(dict(xt=xt, w1g=w1g, w1u=w1u, w2=w2, wr=wr_col))

    nc = _build(cgs)
    return nc, in_maps, assigns, T


def kernel(**inputs) -> np.ndarray:
    nc, in_maps, assigns, T = _stage(inputs)
    res = run_bass_kernel_spmd(nc, in_maps, list(range(NE)))
    out = np.zeros((T, H), np.float32)
    for e in range(NE):
        a, toks = assigns[e]
        if len(a):
            np.add.at(out, toks, res.results[e]["y"][: len(a)])
    return out


# revision 7
# speedup vs baseline: 1.0129x; 1.0129x over previous
"""GPT-OSS MoE experts kernel for Trainium2 (8 NeuronCores, expert-parallel).

Strategy
--------
- Expert-parallel: core e owns expert e's weights (1/8 of total weight bytes).
- Host does routing (gather tokens per expert), weight re-staging (slice the
  expert, transpose to [K, N] contraction-major layout, pad K to a multiple of
  128, cast to fp16), and the final scatter-add combine. No collectives.
- The reference's per-32-block fp8 quant-dequant collapses exactly to
  "round each element to 4 significant bits (RTNE)": the block scale is a
  power of two (mantissa rounding is scale-invariant) and the +-448 clip can
  never bind by construction. Verified numerically; residual differences are
  confined to the e4m3-subnormal range (~2^-9 * block scale, negligible).
  On device this is 3 VectorE ops (Veltkamp split); on host, the same numpy.
  The 4-significant-bit activations are then EXACT in fp16.
- fp16 weights round at 2^-11; end-to-end error vs the f32 reference is
  ~6e-3 absmax-rel (vs ~5e-3 for full fp32/fp22 weights) — the error budget
  is dominated by quantization-boundary flips from layer-1 perturbations
  either way, and fp16 halves the DMA-bound kernel's weight traffic.
- Biases ride free inside the GEMM: contraction padded 2880 -> 2944,
  activations carry a constant-1 row at index 2880, weights the bias row.
- Weight slabs are the moving operand (N=480, full rate), tiny transposed
  activations are the stationary operand, so layer-1 output lands
  token-major: swiglu is pure elementwise work, and the tiny intermediate is
  PE-transposed on chip to feed layer 2.
"""

import functools
import sys

sys.path.insert(0, "/opt/trn_rl_repo")

import numpy as np

import concourse.bass as bass  # noqa: F401
import concourse.mybir as mybir
import concourse.tile as tile
from concourse import bacc
from concourse.bass_utils import run_bass_kernel_spmd

P = 128
H = 2880          # hidden dim (= layer-1 contraction, layer-2 output)
II = 2880         # intermediate dim (gate/up width, layer-2 contraction)
NE = 8            # experts == cores
KT = 23           # k-tiles of 128 over the padded contraction dim
KP = KT * P       # 2944 = 2880 + 64 pad (row 2880 = ones/bias row)
NB = 6            # n-blocks per 2880-wide output
NBW = H // NB     # 480
VC = float(2 ** 20 + 1)            # Veltkamp constant: RTNE to 4 sig bits

f32 = mybir.dt.float32
f16 = mybir.dt.float16
AF = mybir.ActivationFunctionType
ALU = mybir.AluOpType


def _rtne4(x: np.ndarray) -> np.ndarray:
    """Round f32 elements to 4 significant bits, RTNE (== reference
    quant_dequant_fp8 up to e4m3-subnormal leftovers)."""
    c = np.float32(VC)
    t = (x * c).astype(np.float32)
    return (t - (t - x)).astype(np.float32)


@functools.lru_cache(maxsize=4)
def _build(cgs: tuple[int, ...]):
    """Build the per-core Bass program. cgs = token-group sizes (each <=128)."""
    from concourse.masks import make_identity

    nc = bacc.Bacc(None, target_bir_lowering=False)
    ccap = sum(cgs)

    xt_d = nc.declare_dram_parameter("xt", [KP, ccap], f16, isOutput=False)
    w1g_d = nc.declare_dram_parameter("w1g", [KP, II], f16, isOutput=False)
    w1u_d = nc.declare_dram_parameter("w1u", [KP, II], f16, isOutput=False)
    w2_d = nc.declare_dram_parameter("w2", [KP, H], f16, isOutput=False)
    wr_d = nc.declare_dram_parameter("wr", [ccap, 1], f32, isOutput=False)
    y_d = nc.declare_dram_parameter("y", [ccap, H], f32, isOutput=True)

    w1g_r = w1g_d[:].rearrange("(o p) n -> p o n", p=P)
    w1u_r = w1u_d[:].rearrange("(o p) n -> p o n", p=P)
    w2_r = w2_d[:].rearrange("(o p) n -> p o n", p=P)

    with tile.TileContext(nc) as tc:
        with (
            tc.tile_pool(name="consts", bufs=1) as consts,
            tc.tile_pool(name="wslab", bufs=10) as wpool,
            tc.tile_pool(name="tmp", bufs=2) as tmp,
            tc.tile_pool(name="psum", bufs=2, space="PSUM") as psum,
            tc.tile_pool(name="tpsum", bufs=2, space="PSUM") as tpsum,
        ):
            ident = consts.tile([P, P], f32, tag="ident", name="ident")
            make_identity(nc, ident)

            # HAM warmup: dummy matmuls while the first weight slabs stream in,
            # so the real stream starts at the 2.4 GHz clock.
            wup = tpsum.tile([P, P], f32, tag="tp", name="wup")
            for _ in range(48):
                nc.tensor.matmul(wup, ident, ident, start=True, stop=True,
                                 skip_group_check=True)

            # resident per-group tensors
            xts, wrs, inter, interT = [], [], [], []
            col0 = 0
            for g, cg in enumerate(cgs):
                t = consts.tile([P, KT, cg], f16, tag=f"xt{g}", name=f"xt{g}")
                nc.sync.dma_start(
                    t, xt_d[:, col0 : col0 + cg].rearrange("(o p) c -> p o c", p=P)
                )
                xts.append(t)
                w = consts.tile([cg, 1], f32, tag=f"wr{g}", name=f"wr{g}")
                nc.sync.dma_start(w, wr_d[col0 : col0 + cg, :])
                wrs.append(w)
                it = consts.tile([cg, KP], f32, tag=f"inter{g}", name=f"inter{g}")
                nc.vector.memset(it[:, II : II + 1], 1.0)      # ones row (bias)
                nc.vector.memset(it[:, II + 1 :], 0.0)         # K padding
                inter.append(it)
                itT = consts.tile([P, KT, cg], f16, tag=f"interT{g}",
                                  name=f"interT{g}")
                interT.append(itT)
                col0 += cg

            KC = [(0, 12), (12, 23)]   # slab DMA k-chunks

            def load_chunk(src_r, b, ci):
                a, bb = KC[ci]
                t = wpool.tile([P, 12, NBW], f16, tag="wslab", name="wslab")
                nc.sync.dma_start(
                    t[:, : bb - a, :], src_r[:, a:bb, b * NBW : (b + 1) * NBW]
                )
                return t

            def load_slab_pair(src_r1, src_r2, b):
                # interleave the two streams' chunks so the first matmuls only
                # wait on the first chunk of each stream
                g0 = load_chunk(src_r1, b, 0)
                u0 = load_chunk(src_r2, b, 0)
                g1 = load_chunk(src_r1, b, 1)
                u1 = load_chunk(src_r2, b, 1)
                return (g0, g1), (u0, u1)

            def slab_k(slab, k):
                ci = 0 if k < 12 else 1
                return slab[ci][:, k - KC[ci][0], :]

            # ---- layer 1 + swiglu + rtne4 (+ inline transposes) ----
            cg0 = cgs[0]
            ndone = 0   # transposed 128-chunks of inter emitted so far

            def alloc_gu_psums(g, cg):
                # group 0 gets its own banks; later (small) groups share one
                # bank via the column-tiling base-partition trick.
                if g == 0:
                    gps = psum.tile([P, NBW], f32, tag="ps_g0", name="ps_g0")[:cg]
                    ups = psum.tile([P, NBW], f32, tag="ps_u0", name="ps_u0")[:cg]
                else:
                    both = psum.tile([2 * cg, NBW], f32, tag=f"ps_gu{g}",
                                     name=f"ps_gu{g}")
                    gps, ups = both[:cg], both[cg:]
                return gps, ups

            for b in range(NB):
                slab_g, slab_u = load_slab_pair(w1g_r, w1u_r, b)
                for g, cg in enumerate(cgs):
                    gps, ups = alloc_gu_psums(g, cg)
                    for k in range(KT):
                        nc.tensor.matmul(gps, xts[g][:, k, :], slab_k(slab_g, k),
                                         start=(k == 0), stop=(k == KT - 1))
                        nc.tensor.matmul(ups, xts[g][:, k, :], slab_k(slab_u, k),
                                         start=(k == 0), stop=(k == KT - 1))
                    # swiglu: gate=min(G,7); up1=clip(U,-7,7)+1; x=gate*sig(1.702g)*up1
                    gate = tmp.tile([cg0, NBW], f32, tag="t_gate", name="t_gate")[:cg]
                    nc.vector.tensor_scalar_min(gate, gps, 7.0)
                    sig = tmp.tile([cg0, NBW], f32, tag="t_sig", name="t_sig")[:cg]
                    nc.scalar.activation(sig, gate, AF.Sigmoid, scale=1.702)
                    up1 = tmp.tile([cg0, NBW], f32, tag="t_up", name="t_up")[:cg]
                    nc.vector.tensor_scalar(up1, ups, 1.0, -6.0, ALU.add, ALU.max)
                    nc.vector.tensor_scalar_min(up1, up1, 8.0)
                    nc.vector.tensor_mul(gate, gate, sig)          # gate*sig
                    xv = tmp.tile([cg0, NBW], f32, tag="t_xv", name="t_xv")[:cg]
                    nc.vector.tensor_mul(xv, gate, up1)            # x = swiglu
                    tv = tmp.tile([cg0, NBW], f32, tag="t_tv", name="t_tv")[:cg]
                    nc.vector.tensor_scalar_mul(tv, xv, VC)        # t = x*c
                    nc.vector.tensor_sub(xv, tv, xv)               # d = t-x
                    nc.vector.tensor_sub(                          # rtne4 = t-d
                        inter[g][:, b * NBW : (b + 1) * NBW], tv, xv
                    )
                # transpose the 128-chunks completed by this n-block
                ready = ((b + 1) * NBW) // P if b < NB - 1 else KT
                while ndone < ready:
                    i = ndone
                    for g, cg in enumerate(cgs):
                        tp = tpsum.tile([P, cg0], f32, tag="tp", name="tp")[:, :cg]
                        nc.tensor.transpose(
                            tp, inter[g][:, i * P : (i + 1) * P], ident[:cg, :cg]
                        )
                        nc.vector.tensor_copy(interT[g][:, i, :], tp)
                    ndone += 1

            # ---- layer 2 + routing-weight scale ----
            for hb in range(NB):
                slab2 = (load_chunk(w2_r, hb, 0), load_chunk(w2_r, hb, 1))
                row0 = 0
                for g, cg in enumerate(cgs):
                    if g == 0:
                        yps = psum.tile([P, NBW], f32, tag="ps_g0", name="ps_g0")[:cg]
                    else:
                        yps = psum.tile([2 * cg, NBW], f32, tag=f"ps_gu{g}",
                                        name=f"ps_gu{g}")[:cg]
                    for k in range(KT):
                        nc.tensor.matmul(yps, interT[g][:, k, :], slab_k(slab2, k),
                                         start=(k == 0), stop=(k == KT - 1))
                    ysb = tmp.tile([cg0, NBW], f32, tag="t_gate", name="t_gate")[:cg]
                    nc.vector.tensor_scalar_mul(ysb, yps, wrs[g])
                    nc.sync.dma_start(
                        y_d[row0 : row0 + cg, hb * NBW : (hb + 1) * NBW], ysb
                    )
                    row0 += cg

    nc.finalize()
    return nc


def _plan_groups(maxc: int) -> tuple[int, ...]:
    ng = max(1, -(-maxc // P))
    last = maxc - P * (ng - 1)
    last = min(P, max(32, -(-last // 32) * 32))
    return tuple([P] * (ng - 1) + [last])


def _stage(inputs):
    """Host-side routing + weight re-staging. Returns (nc, in_maps, assigns, T)."""
    hs = np.ascontiguousarray(np.asarray(inputs["hidden_states"], dtype=np.float32))
    ri = np.asarray(inputs["router_indices"]).astype(np.int64)
    rw = np.asarray(inputs["routing_weights"], dtype=np.float32)
    gup = np.asarray(inputs["gate_up_proj"], dtype=np.float32)
    gub = np.asarray(inputs["gate_up_proj_bias"], dtype=np.float32)
    dn = np.asarray(inputs["down_proj"], dtype=np.float32)
    dnb = np.asarray(inputs["down_proj_bias"], dtype=np.float32)

    T = hs.shape[0]
    topk = ri.shape[1]

    flat_e = ri.reshape(-1)
    order = np.argsort(flat_e, kind="stable")
    counts = np.bincount(flat_e, minlength=NE)
    starts = np.zeros(NE + 1, np.int64)
    starts[1:] = np.cumsum(counts)
    cgs = _plan_groups(int(counts.max()))
    ccap = sum(cgs)

    x_dq = _rtne4(hs).astype(np.float16)   # 4-sig-bit values: exact in fp16
    rw_flat = rw.reshape(-1)

    in_maps, assigns = [], []
    for e in range(NE):
        a = order[starts[e] : starts[e + 1]]
        toks = a // topk
        ce = len(a)
        assigns.append((a, toks))

        xt = np.zeros((KP, ccap), np.float16)
        xt[:H, :ce] = x_dq[toks].T
        xt[H, :] = np.float16(1.0)

        w1g = np.zeros((KP, II), np.float16)
        w1g[:H, :] = gup[e, 0::2, :].T.astype(np.float16)
        w1g[H, :] = gub[e, 0::2].astype(np.float16)
        w1u = np.zeros((KP, II), np.float16)
        w1u[:H, :] = gup[e, 1::2, :].T.astype(np.float16)
        w1u[H, :] = gub[e, 1::2].astype(np.float16)
        w2 = np.zeros((KP, H), np.float16)
        w2[:II, :] = dn[e].T.astype(np.float16)
        w2[II, :] = dnb[e].astype(np.float16)

        wr_col = np.zeros((ccap, 1), np.float32)
        wr_col[:ce, 0] = rw_flat[a]

        in_maps.append(dict(xt=xt, w1g=w1g, w1u=w1u, w2=w2, wr=wr_col))

    nc = _build(cgs)
    return nc, in_maps, assigns, T


def kernel(**inputs) -> np.ndarray:
    nc, in_maps, assigns, T = _stage(inputs)
    res = run_bass_kernel_spmd(nc, in_maps, list(range(NE)))
    out = np.zeros((T, H), np.float32)
    for e in range(NE):
        a, toks = assigns[e]
        if len(a):
            np.add.at(out, toks, res.results[e]["y"][: len(a)])
    return out
